# revision 35
# baseline (speedup 1.0000x reference)
"""Trainium2 kernel for nn_Contrast: contrastive loss over a 10000x10000
exp-cosine-similarity matrix, sharded by rows across 8 NeuronCores.

Structure:
  host (tiny, O(N*D)): 8->8->8 MLP projection of both views, row norms,
      fold 1/(n1*n2*tau) into the operands:  a = zp1/n1,  b = zp2/(n2*tau).
      Then m[i,j] = exp(a_i . b_j).
  device (O(N^2)), per core k over its 1280-row slice of a:
      main matmuls run in fp8(e4m3) DoubleRow perf mode (2 cols/cycle): the
      K=8 contraction is packed as [4 partitions x 2 interleave].
      The exp of each [128 x gw] PSUM tile is split across two engines by
      column group:
        ACT groups: scalar.activation Exp -> bf16 SBUF tile
        DVE groups: Schraudolph bit-trick exp: (logit*A + B) -> int16,
            bitcast to bf16.  A = 2^7/ln2, B tuned so the mean relative
            error of exp is ~0 (error < 4% per element, averages out in the
            10k-element row/col sums; measured loss rel err ~5e-5).
      Row sums: DVE tensor_scalar (mult 1.0, reduce-add accum_out) over each
      bf16 tile in 4x perf mode -- this frees ACT from accum_out reads.
      Col sums: PE one-hot matmuls (E_c^T @ exp_tile) accumulating into a
      single [20, 512] PSUM bank, bf16.
  host: subtract zero-padding contributions (per-column constant depends on
      which engine produced that column class), add eps, diag from exact
      dots, assemble the two mean log-ratio losses.
"""

import numpy as np

import concourse.bass as bass
import concourse.bacc as bacc
import concourse.mybir as mybir
import concourse.tile as tile
from concourse.bass_utils import run_bass_kernel_spmd

TAU = 0.5
LAM = 0.5
EPS = 1e-8

N = 10000
D = 8
NCORES = 8
RPAD = 10240              # lhs rows padded: 8 cores * 1280
RPC = RPAD // NCORES      # rows per core = 1280
NSTRIP = RPC // 128       # 10 strips of 128 rows
ROW_PAD = RPAD - N        # 240 zero lhs rows

# Schraudolph constants for bf16 bit-pattern exp: v = int16(x*SCH_A + SCH_B),
# bitcast bf16.  SCH_B = 127*2^7 + C with C=-7.5 tuned on the real logit
# distribution for zero mean relative error under the device's
# round-to-nearest float->int conversion (measured: device rowsums sat
# exactly +0.5 LSB above the truncation model).
SCH_A = 128.0 / np.log(2.0)   # 184.6650...
SCH_B = 16256.0 - 7.5

# column tiles cover exactly N columns: 19 x 512 + 272
COL_TILES = [(c * 512, min(512, N - c * 512)) for c in range((N + 511) // 512)]
NCT = len(COL_TILES)      # 20

# Column groups per strip: each group is a list of whole column tiles packed
# into one [128, <=1536] PSUM tensor (3 banks), consumed by one engine:
#   'A' = ACT true exp (accum_out produces the row sums for free),
#   'D' = DVE Schraudolph exp (row sums via a separate DVE 4x pass).
# GPSIMD cannot read PSUM and fp8 DoubleRow tops out at K=128 (both
# rejected by the compiler), so only ACT and DVE can consume matmul output
# and the one-hot colsums stay bf16 per-strip.  Groups must be unions of
# whole 512-aligned column tiles.  Shares balance engine busy: ACT 5904
# cols at ~1.08 ns/elem (incl accum read) ~= DVE 4096 cols at ~1.13 + own
# row sums at 0.26.  Group order interleaves A/D so the two PSUM buffers'
# alternating consumer chains have even latency (~5.9/5.1 us per strip).
GROUP_DEFS = [
    (COL_TILES[9:10] + COL_TILES[18:20], "A", False),   # 512+512+272 = 1296
    (COL_TILES[10:13], "D", False),
    (COL_TILES[0:3], "A", False),
    (COL_TILES[13:16], "D", False),
    (COL_TILES[3:6], "A", False),
    (COL_TILES[16:18], "D", False),
    (COL_TILES[6:9], "A", False),
]
NGROUPS = len(GROUP_DEFS)
assert sum(w for tiles, _, _ in GROUP_DEFS for _, w in tiles) == N

MM_DTYPE = "fp8dr"


def _build_nc(dt_name):
    assert dt_name == "fp8dr"
    f32 = mybir.dt.float32
    bf16 = mybir.dt.bfloat16
    fp8 = mybir.dt.float8e4
    i16 = mybir.dt.int16
    nc = bacc.Bacc(None)

    lhsT = nc.dram_tensor("lhsT", [4, 2, RPC], fp8, kind="ExternalInput")
    rhsT = nc.dram_tensor("rhsT", [4, 2, N], fp8, kind="ExternalInput")
    eblk = nc.dram_tensor("eblk", [128, NCT * 20], bf16, kind="ExternalInput")
    out_rowsum = nc.dram_tensor("out_rowsum", [128, NSTRIP], f32, kind="ExternalOutput")
    out_colsum = nc.dram_tensor("out_colsum", [20, 512], f32, kind="ExternalOutput")

    n_onehot = NSTRIP * NCT

    with tile.TileContext(nc) as tc:
        with (
            tc.tile_pool(name="inp", bufs=1) as inp_pool,
            tc.tile_pool(name="etile", bufs=8) as etile_pool,
            tc.tile_pool(name="rowp", bufs=2) as rowp_pool,
            tc.tile_pool(name="persist", bufs=1) as persist_pool,
            tc.tile_pool(name="pmm", bufs=2, space="PSUM") as pmm_pool,
            tc.tile_pool(name="pcol", bufs=1, space="PSUM") as pcol_pool,
        ):
            lhsT_sb = inp_pool.tile([4, 2, RPC], fp8)
            rhsT_sb = inp_pool.tile([4, 2, N], fp8)
            eblk_sb = inp_pool.tile([128, NCT * 20], bf16)

            # chunked loads: the first strip+group's operands first so
            # compute can start; each dma_start costs ~650ns of sequencer
            # issue, so the tail goes in a few large strided transfers.
            nc.sync.dma_start(
                out=lhsT_sb[:, :, 0:128], in_=lhsT[:, :, 0:128]
            )
            nc.sync.dma_start(
                out=rhsT_sb[:, :, 9216:N], in_=rhsT[:, :, 9216:N]
            )
            nc.sync.dma_start(
                out=rhsT_sb[:, :, 4608:5120], in_=rhsT[:, :, 4608:5120]
            )
            nc.sync.dma_start(out=eblk_sb[:], in_=eblk[:])
            nc.sync.dma_start(
                out=rhsT_sb[:, :, 5120:6656], in_=rhsT[:, :, 5120:6656]
            )
            nc.sync.dma_start(
                out=rhsT_sb[:, :, 0:1536], in_=rhsT[:, :, 0:1536]
            )
            nc.sync.dma_start(
                out=rhsT_sb[:, :, 6656:8192], in_=rhsT[:, :, 6656:8192]
            )
            nc.sync.dma_start(
                out=rhsT_sb[:, :, 1536:3072], in_=rhsT[:, :, 1536:3072]
            )
            nc.sync.dma_start(
                out=lhsT_sb[:, :, 128:RPC], in_=lhsT[:, :, 128:RPC]
            )
            nc.sync.dma_start(
                out=rhsT_sb[:, :, 8192:9216], in_=rhsT[:, :, 8192:9216]
            )
            nc.sync.dma_start(
                out=rhsT_sb[:, :, 3072:4608], in_=rhsT[:, :, 3072:4608]
            )

            rowsum_sb = persist_pool.tile([128, NSTRIP], f32)
            colsum_sb = persist_pool.tile([20, 512], f32)
            # scratch sink for the row-sum pass output: writing et in-place
            # would make the one-hot matmuls (PE) depend on DVE's row-sum
            # queue, convoying PE behind DVE at strip boundaries
            rs_sink = persist_pool.tile([128, 1536], bf16)
            colp = pcol_pool.tile([20, 512], f32)

            # one-hot (column-sum) matmuls are deferred one group behind the
            # exp producers so PE's in-order queue never blocks on them
            pending = []
            onehot_idx = 0

            def flush_one(et, tiles):
                nonlocal onehot_idx
                off = 0
                for c0, w in tiles:
                    c = c0 // 512  # global column-tile index
                    nc.tensor.matmul(
                        colp[:, 0:w],
                        eblk_sb[:, c * 20 : (c + 1) * 20],
                        et[:, off : off + w],
                        start=(onehot_idx == 0),
                        stop=(onehot_idx == n_onehot - 1),
                        skip_group_check=True,
                    )
                    off += w
                    onehot_idx += 1

            def flush_pending(keep=0):
                while len(pending) > keep:
                    flush_one(*pending.pop(0))

            for r in range(NSTRIP):
                rowp = rowp_pool.tile([128, NGROUPS], f32)
                rowsums = []
                for gi, (tiles, who, paired) in enumerate(GROUP_DEFS):
                    gw = sum(w for _, w in tiles)
                    pa = pmm_pool.tile([128, 1536], f32, name=f"pa_{r}_{gi}", tag="pa")
                    # main matmuls: fp8 DoubleRow, 2 cols/cycle
                    off = 0
                    for c0, w in tiles:
                        nc.tensor.matmul(
                            pa[:, off : off + w],
                            lhsT_sb[:, :, r * 128 : (r + 1) * 128],
                            rhsT_sb[:, :, c0 : c0 + w],
                            start=True,
                            stop=True,
                            perf_mode=mybir.MatmulPerfMode.DoubleRow,
                        )
                        off += w
                    et = etile_pool.tile([128, 1536], bf16)
                    if who == "A":
                        # true exp; accum_out gives this group's row sums
                        nc.scalar.activation(
                            et[:, :gw],
                            pa[:, :gw],
                            mybir.ActivationFunctionType.Exp,
                            accum_out=rowp[:, gi : gi + 1],
                        )
                    else:
                        # Schraudolph exp on DVE, emitted in group order so
                        # it frees the PSUM buffer as early as possible
                        nc.vector.tensor_scalar(
                            out=et[:, :gw].bitcast(i16),
                            in0=pa[:, :gw],
                            scalar1=float(SCH_A),
                            scalar2=float(SCH_B),
                            op0=mybir.AluOpType.mult,
                            op1=mybir.AluOpType.add,
                        )
                        rowsums.append((et, gw, gi))
                    flush_pending(keep=1)
                    pending.append((et, tiles))
                # row-sums of the D groups: DVE 4x-mode read-only passes
                # with reduce accum, deferred to the end of the strip so
                # they never delay the Schraudolph passes
                for et, gw, gi in rowsums:
                    nc.vector.tensor_scalar(
                        out=rs_sink[:, :gw],
                        in0=et[:, :gw],
                        scalar1=1.0,
                        scalar2=None,
                        op0=mybir.AluOpType.mult,
                        op1=mybir.AluOpType.add,
                        accum_out=rowp[:, gi : gi + 1],
                    )
                nc.vector.reduce_sum(
                    out=rowsum_sb[:, r : r + 1],
                    in_=rowp[:, :],
                    axis=mybir.AxisListType.X,
                )
            flush_pending()

            nc.vector.tensor_copy(out=colsum_sb[:], in_=colp[:])
            nc.sync.dma_start(out=out_rowsum[:], in_=rowsum_sb[:])
            nc.sync.dma_start(out=out_colsum[:], in_=colsum_sb[:])

    nc.compile()
    return nc


_NC_CACHE = {}


def _get_nc(dt_name):
    if dt_name not in _NC_CACHE:
        _NC_CACHE[dt_name] = _build_nc(dt_name)
    return _NC_CACHE[dt_name]


def _proj_np(z, W1, b1, W2, b2):
    h = z @ W1.T + b1
    h = np.where(h > 0, h, np.expm1(h)).astype(np.float32)
    return (h @ W2.T + b2).astype(np.float32)


def _prepare_operands(z_mp, z_sc, W1, b1, W2, b2):
    zp1 = _proj_np(z_mp.astype(np.float32), W1, b1, W2, b2)
    zp2 = _proj_np(z_sc.astype(np.float32), W1, b1, W2, b2)
    n1 = np.sqrt(np.sum(zp1 * zp1, axis=1, keepdims=True)).astype(np.float32)
    n2 = np.sqrt(np.sum(zp2 * zp2, axis=1, keepdims=True)).astype(np.float32)
    a = (zp1 / n1).astype(np.float32)
    b = (zp2 / (n2 * np.float32(TAU))).astype(np.float32)
    dots = np.sum(a * b, axis=1).astype(np.float32)  # diag logits (exact path)
    return a, b, dots


def _pack_dr(mat_T):
    """[8, X] contraction-major -> fp8 [4, 2, X] DoubleRow layout with
    k(p, i) = i*4 + p."""
    import ml_dtypes

    f8 = ml_dtypes.float8_e4m3
    return np.ascontiguousarray(
        mat_T.reshape(2, 4, mat_T.shape[1]).transpose(1, 0, 2)
    ).astype(f8)


def _make_in_maps(a, b):
    import ml_dtypes

    bf16 = ml_dtypes.bfloat16
    f8 = ml_dtypes.float8_e4m3
    a_pad = np.zeros((RPAD, D), np.float32)
    a_pad[:N] = a
    aT = np.ascontiguousarray(a_pad.T)          # [8, RPAD]
    bT4 = _pack_dr(np.ascontiguousarray(b.T))   # [4, 2, N] fp8
    E = np.ascontiguousarray(
        np.tile(np.eye(20, dtype=bf16)[None], (128, 1, 1)).reshape(128, NCT * 20)
    )
    return [
        {
            "lhsT": _pack_dr(aT[:, k * RPC : (k + 1) * RPC]),
            "rhsT": bT4,
            "eblk": E,
        }
        for k in range(NCORES)
    ]


def _col_pad_constants():
    """Per-column constant contributed by each zero-padded lhs row: ACT
    columns get exp(0)=1; Schraudolph columns get the bf16 whose bits are
    round(SCH_B); paired columns additionally go through the bf16->fp8
    round-to-nearest copy."""
    import ml_dtypes

    v0b = np.round(np.float32(SCH_B)).astype(np.int16).view(ml_dtypes.bfloat16)
    adj = np.empty(N, np.float64)
    for tiles, who, paired in GROUP_DEFS:
        v = np.float32(1.0) if who == "A" else v0b.astype(np.float32)
        if paired:
            v = v.astype(ml_dtypes.float8_e4m3).astype(np.float32)
        for c0, w in tiles:
            adj[c0 : c0 + w] = np.float64(v)
    return adj


def _finalize(res, dots):
    rowsum_full = np.concatenate(
        [np.asarray(res[k]["out_rowsum"]).T.reshape(-1) for k in range(NCORES)]
    )
    colsum_full = np.sum(
        [np.asarray(res[k]["out_colsum"]).reshape(-1) for k in range(NCORES)], axis=0
    )
    row_sum = rowsum_full[:N].astype(np.float64) + EPS
    col_sum = (
        colsum_full[:N].astype(np.float64) - ROW_PAD * _col_pad_constants() + EPS
    )
    diag = np.exp(dots.astype(np.float64))
    lori_mp = -np.mean(np.log(diag / row_sum))
    lori_sc = -np.mean(np.log(diag / col_sum))
    return np.float32(LAM * lori_mp + (1.0 - LAM) * lori_sc)


def kernel(z_mp, z_sc, W1, b1, W2, b2):
    a, b, dots = _prepare_operands(z_mp, z_sc, W1, b1, W2, b2)
    in_maps = _make_in_maps(a, b)
    nc = _get_nc(MM_DTYPE)
    res = run_bass_kernel_spmd(nc, in_maps, list(range(NCORES))).results
    return _finalize(res, dots)
